# revision 29
# baseline (speedup 1.0000x reference)
"""Trainium2 Bass kernel for the DNL (disentangled non-local) attention block.

Reference computation (per batch b, with xf = x.reshape(B, C, N), N = H*W):
    q  = (wq @ xf + bq)  centered over n          [N, 32]
    k  = (wk @ xf + bk)  centered over n          [32, N]
    A  = softmax_rows(q @ k)                      [N, N]
    v  = relu(wv @ xf + bv)                       [C, N]
    mask = softmax(wm @ xf + bm)                  [N]
    tissue[c, m] = sum_n v[c, n] * (A[m, n] + mask[n])
    return (x, tissue)

Math simplifications used (all exact):
  - q/k biases, bm, and k-centering add per-row constants inside the row
    softmax and drop out; only q-centering survives (as "-mean_n q").
  - The mask term is a rank-1 correction vm[c] = sum_n v[c,n] mask[n].
  - No max-subtraction in softmax: |energy| <= ~5 for these input scales.

Device layout (per core; 8 cores = 4 batches x 2 query-halves of 2048):
  - Projections and attention are a single software-pipelined phase: the
    V/pm projection loop interleaves the E^T matmul + exp for m-chunks
    0-1 (exp run-ahead on a deep fp8 e_sb ring), and chunks 2-3's E+exp
    ride under chunks 0-1's AV matmuls, so the Act engine (exp is the
    per-engine floor at ~55us) never waits on a projection phase.
  - q and k are one packed stationary [wq|wk]; E^T runs fp8 DoubleRow
    (k-tiles of 16) with two j-blocks row-packed at PE row tiles 0/32.
  - exp(E^T) is fp8e4; the AV matmul is fp8 DoubleRow (2 j-blocks =
    K=256 per pass).  The V^T fp8 stationary carries a 257th all-ones
    column: the softmax denominator colsum falls out of the AV passes.
  - Host-side weight scaling keeps fp8 in the normal range: wq,wk x4
    (energy x16, undone by exp scale=1/16), wv x16 (undone via the 1/s
    broadcast row = 1/16 and rz).  wv itself stays bf16: weight-quant
    error does not average out over the attention sum.
  - bv is added by the Vector engine (broadcast tile), not a PE pass.
  - The pm (mask logit) projection rides the V projection as column 256
    of the wvm stationary (pre-bias; bm drops out of its softmax).
  - The per-core query half is selected by permuting the spatial columns
    of the input on the host (j-sums are permutation invariant).
"""

import sys

import numpy as np

if "/opt/trn_rl_repo" not in sys.path:
    sys.path.insert(0, "/opt/trn_rl_repo")

import concourse.bacc as bacc
import concourse.bass as bass
import concourse.mybir as mybir
import concourse.tile as tile
from concourse.bass_utils import run_bass_kernel_spmd

F32 = mybir.dt.float32
F32R = mybir.dt.float32r
BF16 = mybir.dt.bfloat16
FP8 = mybir.dt.float8e4
AF = mybir.ActivationFunctionType
DR = mybir.MatmulPerfMode.DoubleRow

B, C, H, W = 4, 256, 64, 64
N = H * W          # 4096 spatial positions
D = 32             # C // 8, q/k channel dim
M = N // 2         # query rows per core (2048)
NB = N // 128      # 32 j-blocks
NG = NB // 2       # 16 j-block pairs (DoubleRow consumes 2 blocks/pass)
NMC = M // 512     # 4 m-chunks per core
CP = 272           # vt free width: 256 ch + ones col @256, padded to a
                   # 16B-aligned DoubleRow k-tile stride (fp8 Ldweights ISA)
QS = 4.0           # host scale on wq and wk (energy x16)
VS = 16.0          # host scale on wv/bv/wm (v and pm x16)
RS = 1.0 / VS
N_CORES = 8


def build_nc():
    nc = bacc.Bacc("TRN2", target_bir_lowering=False)

    x_d = nc.dram_tensor("x", [128, 2, N], BF16, kind="ExternalInput")
    wqkt_d = nc.dram_tensor("wqkt", [128, 2, 2 * D], BF16, kind="ExternalInput")
    wvmt_d = nc.dram_tensor("wvmt", [128, 2, CP], BF16, kind="ExternalInput")
    bvm_d = nc.dram_tensor("bvm", [1, CP], BF16, kind="ExternalInput")
    out_d = nc.dram_tensor("out", [C, M], F32, kind="ExternalOutput")

    with tile.TileContext(nc) as tc, nc.allow_low_precision(
        reason="bf16/fp8 matmul operands are a deliberate precision/speed trade"
    ):
        with (
            tc.tile_pool(name="const", bufs=1) as cpool,
            tc.tile_pool(name="work", bufs=1) as wpool,
            tc.tile_pool(name="vwork", bufs=2) as vpool,
            tc.tile_pool(name="norm", bufs=2) as npool,
            tc.tile_pool(name="expsb", bufs=36) as epool,
            tc.tile_pool(name="osb", bufs=2) as opool,
        ):
            # ---------------- loads + constants ----------------
            wqkt = cpool.tile([128, 2, 2 * D], BF16, tag="wqkt")
            wvmt = cpool.tile([128, 2, CP], BF16, tag="wvmt")
            bvm = cpool.tile([1, CP], BF16, tag="bvm")
            nc.sync.dma_start(wqkt[:], wqkt_d[:])
            nc.sync.dma_start(wvmt[:], wvmt_d[:])
            nc.sync.dma_start(bvm[:], bvm_d[:])

            xsb = cpool.tile([128, 2, N], BF16, tag="xsb")
            for t in range(8):
                nc.sync.dma_start(
                    xsb[:, :, bass.ts(t, 512)], x_d[:, :, bass.ts(t, 512)]
                )

            ones_colf = cpool.tile([128, 1], F32, tag="ones_colf")
            ones_rowf = cpool.tile([1, 128], F32, tag="ones_rowf")
            nc.vector.memset(ones_colf[:], 1.0)
            nc.vector.memset(ones_rowf[:], 1.0)
            ones_row_b = cpool.tile([1, 128], BF16, tag="ones_row_b")
            ones_col_8 = cpool.tile([128, 1], FP8, tag="ones_col_8")
            nc.vector.tensor_copy(ones_row_b[:], ones_rowf[:])
            nc.vector.tensor_copy(ones_col_8[:], ones_colf[:])
            # 1/s broadcast stationary carries the 1/VS unscale of vt
            rsc_f = cpool.tile([1, 128], F32, tag="rsc_f")
            nc.vector.memset(rsc_f[:], RS)
            rsc_row = cpool.tile([1, 128], F32R, tag="rsc_row")
            nc.vector.tensor_copy(rsc_row[:], rsc_f[:])

            # k/q in fp8 DoubleRow layout [16 x 2 k-tiles], replicated to
            # partition group 32 for 2-way row-packed E^T matmuls
            qt_sb = wpool.tile([D, N], F32, tag="qt_sb")
            qacc = wpool.tile([D, 8], F32, tag="qacc")
            kt64 = cpool.tile([64, N], FP8, tag="kt64")
            qtmp = cpool.tile([D, M], FP8, tag="qtmp")
            k8 = cpool.tile([64, 2, N], FP8, tag="k8")
            q8 = cpool.tile([64, 2, M], FP8, tag="q8")
            vt_sb = cpool.tile([128, NB, CP], FP8, tag="vt_sb")
            bvb = cpool.tile([128, CP], BF16, tag="bvb")
            pmt_col = cpool.tile([128, NB], F32, tag="pmt_col")
            epm_col = cpool.tile([128, NB, 16], FP8, tag="epm_col")
            vm_col = cpool.tile([128, 2], F32, tag="vm_col")

            # ones column of V^T (softmax denominator rides the AV matmul)
            nc.vector.memset(vt_sb[:, :, 256:257], 1.0)

            with tc.tile_pool(name="psE", bufs=2, space="PSUM") as psE:

                def emit_e(mc, g):
                    # two K=16x2 fp8-DR matmuls packed at PE row tiles 0/32
                    e_ps = psE.tile([128, 2, 512], F32, tag="e_ps", name="e_ps")
                    for h in range(2):
                        jb = 2 * g + h
                        nc.tensor.matmul(
                            e_ps[:, h, :],
                            k8[h * 32 : h * 32 + 16, :, bass.ts(jb, 128)],
                            q8[h * 32 : h * 32 + 16, :, bass.ts(mc, 512)],
                            start=True,
                            stop=True,
                            perf_mode=DR,
                            tile_position=(h * 32, 0),
                        )
                    return e_ps

                def do_exp(e_ps):
                    e_sb = epool.tile([128, 2, 512], FP8, tag="e_sb",
                                      name="e_sb")
                    # scale=1/16 undoes the host x4 on wq and wk
                    nc.scalar.activation(
                        e_sb[:], e_ps[:], AF.Exp, scale=1.0 / 16.0
                    )
                    return e_sb

                # ---- packed q|k projection: one stationary, 16 matmuls ----
                with tc.tile_pool(name="psA", bufs=2, space="PSUM") as psA:
                    for t in range(8):
                        kqp = psA.tile([2 * D, 512], F32, tag="kq_ps")
                        for cb in range(2):
                            nc.tensor.matmul(
                                kqp[:],
                                wqkt[:, cb, :],
                                xsb[:, cb, bass.ts(t, 512)],
                                start=(cb == 0),
                                stop=(cb == 1),
                            )
                        nc.scalar.activation(
                            qt_sb[:, bass.ts(t, 512)], kqp[0:D, :], AF.Copy,
                            accum_out=qacc[:, t : t + 1],
                        )
                        nc.vector.tensor_copy(
                            kt64[D : 2 * D, bass.ts(t, 512)], kqp[D : 2 * D, :]
                        )

                    # center q over n:  qc = q - mean_n(q); first M cols used
                    qsum = wpool.tile([D, 1], F32, tag="qsum")
                    nc.vector.reduce_sum(
                        qsum[:], qacc[:], axis=mybir.AxisListType.X
                    )
                    qneg = wpool.tile([D, 1], F32, tag="qneg")
                    nc.scalar.mul(qneg[:], qsum[:], -1.0 / N)
                    nc.scalar.activation(
                        qtmp[:], qt_sb[:, 0:M], AF.Identity, bias=qneg[:, 0:1]
                    )
                    # scatter k-tiles into DR layout; replicas read the
                    # same sources directly so all transfers are depth-1
                    nc.sync.dma_start(k8[0:16, 0, :], kt64[32:48, :])
                    nc.sync.dma_start(k8[0:16, 1, :], kt64[48:64, :])
                    nc.sync.dma_start(q8[0:16, 0, :], qtmp[0:16, :])
                    nc.sync.dma_start(q8[0:16, 1, :], qtmp[16:32, :])
                    nc.sync.dma_start(k8[32:48, 0, :], kt64[32:48, :])
                    nc.sync.dma_start(k8[32:48, 1, :], kt64[48:64, :])
                    nc.sync.dma_start(q8[32:48, 0, :], qtmp[0:16, :])
                    nc.sync.dma_start(q8[32:48, 1, :], qtmp[16:32, :])

                # ---- chunk 0: V/pm projection rides just-in-time inside
                # ---- the AV loop; vt pair g is produced right before use
                with (
                    tc.tile_pool(name="psO", bufs=1, space="PSUM") as psO,
                    tc.tile_pool(name="psS", bufs=1, space="PSUM") as psS,
                ):

                    def make_acc():
                        o_ps = [
                            psO.tile([128, 512], F32, tag=f"o_ps{ci}",
                                     name=f"o_ps{ci}")
                            for ci in range(2)
                        ]
                        s_ps = psS.tile([1, 512], F32, tag="s_ps", name="s_ps")
                        return o_ps, s_ps

                    def av_group(o_ps, s_ps, g, e_sb):
                        first = g == 0
                        last = g == NG - 1
                        nc.tensor.matmul(
                            s_ps[:],
                            vt_sb[:, 2 * g : 2 * g + 2, 256:257],
                            e_sb[:],
                            start=first,
                            stop=last,
                            perf_mode=DR,
                        )
                        for ci in range(2):
                            nc.tensor.matmul(
                                o_ps[ci][:],
                                vt_sb[:, 2 * g : 2 * g + 2,
                                      128 * ci : 128 * (ci + 1)],
                                e_sb[:],
                                start=first,
                                stop=last,
                                perf_mode=DR,
                            )

                    def tail(mc, o_ps, s_ps, psR):
                        # normalize: out[c,m] = O[c,m] / (VS*s[m]) + vm[c]
                        rs_f = npool.tile([1, 512], F32, tag="rs_f")
                        rs_scr = npool.tile([1, 512], F32, tag="rs_scr")
                        nc.vector.reciprocal_approx_accurate(
                            rs_f[:], s_ps[:], rs_scr[:]
                        )
                        rs_row = npool.tile([1, 512], F32R, tag="rs_row")
                        nc.vector.tensor_copy(rs_row[:], rs_f[:])
                        rb_ps = psR.tile([128, 512], F32, tag="rb_ps",
                                         name="rb_ps")
                        nc.tensor.matmul(
                            rb_ps[:], rsc_row[:], rs_row[:],
                            start=True, stop=True,
                        )
                        rb_sb = npool.tile([128, 512], F32, tag="rb_sb")
                        nc.vector.tensor_copy(rb_sb[:], rb_ps[:])
                        for ci in range(2):
                            t_sb = opool.tile([128, 512], F32, tag="t_sb",
                                              name="t_sb")
                            nc.vector.tensor_mul(
                                t_sb[:], o_ps[ci][:], rb_sb[:]
                            )
                            o_sb = opool.tile([128, 512], F32, tag="o_sb",
                                              name="o_sb")
                            nc.vector.tensor_scalar_add(
                                o_sb[:], t_sb[:], vm_col[:, ci : ci + 1]
                            )
                            nc.sync.dma_start(
                                out_d[128 * ci : 128 * (ci + 1),
                                      bass.ts(mc, 512)],
                                o_sb[:],
                            )

                    o_ps0, s_ps0 = make_acc()
                    with tc.tile_pool(name="psB", bufs=1, space="PSUM") as psB:
                        # bias broadcast tile (bv is added on the Vector
                        # engine)
                        bvb_ps = psB.tile([128, CP], F32, tag="v_ps",
                                          name="bvb")
                        nc.tensor.matmul(
                            bvb_ps[:, 0:257], ones_row_b[:], bvm[:, 0:257],
                            start=True, stop=True,
                        )
                        nc.vector.tensor_copy(bvb[:], bvb_ps[:])

                        for g in range(NG):
                            for h in range(2):
                                jb = 2 * g + h
                                vp = psB.tile([128, CP], F32, tag="v_ps",
                                              name="v_ps")
                                for cb in range(2):
                                    nc.tensor.matmul(
                                        vp[:, 0:257],
                                        xsb[:, cb, bass.ts(jb, 128)],
                                        wvmt[:, cb, 0:257],
                                        start=(cb == 0),
                                        stop=(cb == 1),
                                    )
                                # pm column is pre-bias (bm drops out)
                                nc.scalar.copy(
                                    pmt_col[:, jb : jb + 1], vp[:, 256:257]
                                )
                                vtmp = vpool.tile([128, 256], F32, tag="vtmp",
                                                  name="vtmp")
                                nc.vector.tensor_add(
                                    vtmp[:], vp[:, 0:256], bvb[:, 0:256]
                                )
                                nc.vector.tensor_scalar_max(
                                    vt_sb[:, jb, 0:256], vtmp[:], 0.0
                                )
                            av_group(o_ps0, s_ps0, g, do_exp(emit_e(0, g)))

                    # chunk 1's first energy: keep the Act queue fed while
                    # the mask path runs
                    e_pending = emit_e(1, 0)

                    # ---- mask path: vm[c] = sum_n V^T[n,c] epm[n]/(VS*z) ----
                    with tc.tile_pool(name="psSa", bufs=1,
                                      space="PSUM") as psSa:
                        nc.scalar.activation(
                            epm_col[:, :, 0], pmt_col[:], AF.Exp, scale=RS
                        )
                        sa1 = psSa.tile([128, 512], F32, tag="sa",
                                        name="sa_z")
                        zp = sa1[0:1, 0:NB]
                        nc.tensor.matmul(
                            zp, ones_col_8[:], epm_col[:, :, 0],
                            start=True, stop=True,
                        )
                        zt = wpool.tile([1, 1], F32, tag="zt")
                        nc.vector.reduce_sum(
                            zt[:], zp, axis=mybir.AxisListType.X
                        )
                        rz = wpool.tile([1, 1], F32, tag="rz")
                        nc.vector.reciprocal(rz[:], zt[:])

                        sa2 = psSa.tile([128, 512], F32, tag="sa",
                                        name="sa_vm")
                        vmp = sa2[0:1, 0:C]
                        for g in range(NG):
                            nc.tensor.matmul(
                                vmp,
                                epm_col[:, 2 * g : 2 * g + 2, 0:1],
                                vt_sb[:, 2 * g : 2 * g + 2, 0:256],
                                start=(g == 0),
                                stop=(g == NG - 1),
                                perf_mode=DR,
                            )
                        vm_row = wpool.tile([1, C], F32, tag="vm_row")
                        nc.vector.tensor_scalar(
                            vm_row[:], vmp, rz[0:1, 0:1], RS,
                            mybir.AluOpType.mult, mybir.AluOpType.mult,
                        )
                        sa3 = psSa.tile([128, 512], F32, tag="sa",
                                        name="sa_vc")
                        vcp = sa3[:, 0:2]
                        for ci in range(2):
                            nc.tensor.transpose(
                                vcp[:, ci : ci + 1],
                                vm_row[0:1, 128 * ci : 128 * (ci + 1)],
                                ones_rowf[0:1, 0:1],
                            )
                        nc.vector.tensor_copy(vm_col[:], vcp[:])

                    # ---- tails + software-pipelined chunks 1-3 ----
                    with tc.tile_pool(name="psR", bufs=1,
                                      space="PSUM") as psR:
                        tail(0, o_ps0, s_ps0, psR)
                        for mc in range(1, NMC):
                            o_ps, s_ps = make_acc()
                            for g in range(NG):
                                e_sb = do_exp(e_pending)
                                if g + 1 < NG:
                                    e_pending = emit_e(mc, g + 1)
                                elif mc + 1 < NMC:
                                    e_pending = emit_e(mc + 1, 0)
                                av_group(o_ps, s_ps, g, e_sb)
                            tail(mc, o_ps, s_ps, psR)

    nc.compile()
    return nc


_NC_CACHE = {}


def _get_nc():
    if "nc" not in _NC_CACHE:
        _NC_CACHE["nc"] = build_nc()
    return _NC_CACHE["nc"]


def build_in_maps(x, wq, bq, wk, bk, wv, bv, wm, bm):
    import ml_dtypes

    bf16 = ml_dtypes.bfloat16
    x = np.ascontiguousarray(np.asarray(x, dtype=np.float32))
    xf = x.reshape(B, C, N)

    def blocked(a, dt):
        # [C, F] -> [128, 2, F] (channel block index in dim 1)
        f = a.shape[1]
        return np.ascontiguousarray(
            a.reshape(2, 128, f).transpose(1, 0, 2).astype(dt)
        )

    wqk = np.concatenate(
        [np.asarray(wq, np.float32).T * QS, np.asarray(wk, np.float32).T * QS],
        axis=1,
    )  # [C, 64]: q cols 0-31, k cols 32-63
    wqkt = blocked(wqk, bf16)
    wvm = np.concatenate(
        [
            np.asarray(wv, np.float32).T * VS,
            np.asarray(wm, np.float32).T * VS,
            np.zeros((C, CP - 257), np.float32),
        ],
        axis=1,
    )  # [C, CP]
    wvmt = blocked(wvm, bf16)
    bvm = np.concatenate(
        [np.asarray(bv, np.float32).reshape(C) * VS, np.zeros(CP - C, np.float32)]
    ).reshape(1, CP).astype(bf16)

    in_maps = []
    for core in range(N_CORES):
        b, half = divmod(core, 2)
        if half == 0:
            xin = xf[b]
        else:
            # own query half first; j-sums are permutation invariant
            xin = np.concatenate([xf[b][:, M:], xf[b][:, :M]], axis=1)
        xin = np.ascontiguousarray(
            xin.reshape(2, 128, N).transpose(1, 0, 2).astype(bf16)
        )
        in_maps.append(
            {
                "x": xin,
                "wqkt": wqkt,
                "wvmt": wvmt,
                "bvm": bvm,
            }
        )
    return x, in_maps


def kernel(x, wq, bq, wk, bk, wv, bv, wm, bm):
    x, in_maps = build_in_maps(x, wq, bq, wk, bk, wv, bv, wm, bm)

    res = run_bass_kernel_spmd(_get_nc(), in_maps, list(range(N_CORES)))
    _NC_CACHE["last_results"] = res

    tissue = np.empty((B, C, N), np.float32)
    for core in range(N_CORES):
        b, half = divmod(core, 2)
        tissue[b][:, half * M : (half + 1) * M] = res.results[core]["out"]
    return x, tissue.reshape(B, C, H, W)


# revision 31
# speedup vs baseline: 1.0311x; 1.0311x over previous
"""Trainium2 Bass kernel for the DNL (disentangled non-local) attention block.

Reference computation (per batch b, with xf = x.reshape(B, C, N), N = H*W):
    q  = (wq @ xf + bq)  centered over n          [N, 32]
    k  = (wk @ xf + bk)  centered over n          [32, N]
    A  = softmax_rows(q @ k)                      [N, N]
    v  = relu(wv @ xf + bv)                       [C, N]
    mask = softmax(wm @ xf + bm)                  [N]
    tissue[c, m] = sum_n v[c, n] * (A[m, n] + mask[n])
    return (x, tissue)

Math simplifications used (all exact):
  - q/k biases, bm, and k-centering add per-row constants inside the row
    softmax and drop out; only q-centering survives (as "-mean_n q").
  - The mask term is a rank-1 correction vm[c] = sum_n v[c,n] mask[n].
  - No max-subtraction in softmax: |energy| <= ~5 for these input scales.

Device layout (per core; 8 cores = 4 batches x 2 query-halves of 2048):
  - Projections and attention are a single software-pipelined phase: the
    V/pm projection loop interleaves the E^T matmul + exp for m-chunks
    0-1 (exp run-ahead on a deep fp8 e_sb ring), and chunks 2-3's E+exp
    ride under chunks 0-1's AV matmuls, so the Act engine (exp is the
    per-engine floor at ~55us) never waits on a projection phase.
  - q and k are one packed stationary [wq|wk]; E^T runs fp8 DoubleRow
    (k-tiles of 16) with two j-blocks row-packed at PE row tiles 0/32.
  - exp(E^T) is fp8e4; the AV matmul is fp8 DoubleRow (2 j-blocks =
    K=256 per pass).  The V^T fp8 stationary carries a 257th all-ones
    column: the softmax denominator colsum falls out of the AV passes.
  - Host-side weight scaling keeps fp8 in the normal range: wq,wk x4
    (energy x16, undone by exp scale=1/16), wv x16 (undone via the 1/s
    broadcast row = 1/16 and rz).  wv itself stays bf16: weight-quant
    error does not average out over the attention sum.
  - bv is added by the Vector engine (broadcast tile), not a PE pass.
  - The pm (mask logit) projection rides the V projection as column 256
    of the wvm stationary (pre-bias; bm drops out of its softmax).
  - The per-core query half is selected by permuting the spatial columns
    of the input on the host (j-sums are permutation invariant).
"""

import sys

import numpy as np

if "/opt/trn_rl_repo" not in sys.path:
    sys.path.insert(0, "/opt/trn_rl_repo")

import concourse.bacc as bacc
import concourse.bass as bass
import concourse.mybir as mybir
import concourse.tile as tile
from concourse.bass_utils import run_bass_kernel_spmd

F32 = mybir.dt.float32
F32R = mybir.dt.float32r
BF16 = mybir.dt.bfloat16
FP8 = mybir.dt.float8e4
AF = mybir.ActivationFunctionType
DR = mybir.MatmulPerfMode.DoubleRow

B, C, H, W = 4, 256, 64, 64
N = H * W          # 4096 spatial positions
D = 32             # C // 8, q/k channel dim
M = N // 2         # query rows per core (2048)
NB = N // 128      # 32 j-blocks
NG = NB // 2       # 16 j-block pairs (DoubleRow consumes 2 blocks/pass)
NMC = M // 512     # 4 m-chunks per core
CP = 272           # vt free width: 256 ch + ones col @256, padded to a
                   # 16B-aligned DoubleRow k-tile stride (fp8 Ldweights ISA)
QS = 4.0           # host scale on wq and wk (energy x16)
VS = 16.0          # host scale on wv/bv/wm (v and pm x16)
RS = 1.0 / VS
N_CORES = 8


def build_nc():
    nc = bacc.Bacc("TRN2", target_bir_lowering=False)

    x_d = nc.dram_tensor("x", [128, 2, N], BF16, kind="ExternalInput")
    wqkt_d = nc.dram_tensor("wqkt", [128, 2, 2 * D], BF16, kind="ExternalInput")
    wvmt_d = nc.dram_tensor("wvmt", [128, 2, CP], BF16, kind="ExternalInput")
    bvm_d = nc.dram_tensor("bvm", [1, CP], BF16, kind="ExternalInput")
    out_d = nc.dram_tensor("out", [C, M], F32, kind="ExternalOutput")

    with tile.TileContext(nc) as tc, nc.allow_low_precision(
        reason="bf16/fp8 matmul operands are a deliberate precision/speed trade"
    ):
        with (
            tc.tile_pool(name="const", bufs=1) as cpool,
            tc.tile_pool(name="work", bufs=1) as wpool,
            tc.tile_pool(name="vwork", bufs=2) as vpool,
            tc.tile_pool(name="norm", bufs=2) as npool,
            tc.tile_pool(name="expsb", bufs=36) as epool,
            tc.tile_pool(name="osb", bufs=2) as opool,
        ):
            # ---------------- loads + constants ----------------
            wqkt = cpool.tile([128, 2, 2 * D], BF16, tag="wqkt")
            wvmt = cpool.tile([128, 2, CP], BF16, tag="wvmt")
            bvm = cpool.tile([1, CP], BF16, tag="bvm")
            nc.sync.dma_start(wqkt[:], wqkt_d[:])
            nc.sync.dma_start(wvmt[:], wvmt_d[:])
            nc.sync.dma_start(bvm[:], bvm_d[:])

            xsb = cpool.tile([128, 2, N], BF16, tag="xsb")
            for t in range(8):
                nc.sync.dma_start(
                    xsb[:, :, bass.ts(t, 512)], x_d[:, :, bass.ts(t, 512)]
                )

            ones_colf = cpool.tile([128, 1], F32, tag="ones_colf")
            ones_rowf = cpool.tile([1, 128], F32, tag="ones_rowf")
            nc.vector.memset(ones_colf[:], 1.0)
            nc.vector.memset(ones_rowf[:], 1.0)
            ones_row_b = cpool.tile([1, 128], BF16, tag="ones_row_b")
            ones_col_8 = cpool.tile([128, 1], FP8, tag="ones_col_8")
            nc.vector.tensor_copy(ones_row_b[:], ones_rowf[:])
            nc.vector.tensor_copy(ones_col_8[:], ones_colf[:])
            # 1/s broadcast stationary carries the 1/VS unscale of vt
            rsc_f = cpool.tile([1, 128], F32, tag="rsc_f")
            nc.vector.memset(rsc_f[:], RS)
            rsc_row = cpool.tile([1, 128], F32R, tag="rsc_row")
            nc.vector.tensor_copy(rsc_row[:], rsc_f[:])

            # k/q in fp8 DoubleRow layout [16 x 2 k-tiles], replicated to
            # partition group 32 for 2-way row-packed E^T matmuls
            qt_sb = wpool.tile([D, N], F32, tag="qt_sb")
            qacc = wpool.tile([D, 8], F32, tag="qacc")
            kt64 = cpool.tile([64, N], FP8, tag="kt64")
            qtmp = cpool.tile([D, M], FP8, tag="qtmp")
            k8 = cpool.tile([64, 2, N], FP8, tag="k8")
            q8 = cpool.tile([64, 2, M], FP8, tag="q8")
            vt_sb = cpool.tile([128, NB, CP], FP8, tag="vt_sb")
            bvb = cpool.tile([128, CP], BF16, tag="bvb")
            pmt_col = cpool.tile([128, NB], F32, tag="pmt_col")
            epm_col = cpool.tile([128, NB, 16], FP8, tag="epm_col")
            vm_col = cpool.tile([128, 2], F32, tag="vm_col")

            # ones column of V^T (softmax denominator rides the AV matmul)
            nc.vector.memset(vt_sb[:, :, 256:257], 1.0)

            with tc.tile_pool(name="psE", bufs=2, space="PSUM") as psE:

                def emit_e(mc, g):
                    # two K=16x2 fp8-DR matmuls packed at PE row tiles 0/32
                    e_ps = psE.tile([128, 2, 512], F32, tag="e_ps", name="e_ps")
                    for h in range(2):
                        jb = 2 * g + h
                        nc.tensor.matmul(
                            e_ps[:, h, :],
                            k8[h * 32 : h * 32 + 16, :, bass.ts(jb, 128)],
                            q8[h * 32 : h * 32 + 16, :, bass.ts(mc, 512)],
                            start=True,
                            stop=True,
                            perf_mode=DR,
                            tile_position=(h * 32, 0),
                        )
                    return e_ps

                def do_exp(e_ps):
                    e_sb = epool.tile([128, 2, 512], FP8, tag="e_sb",
                                      name="e_sb")
                    # scale=1/16 undoes the host x4 on wq and wk
                    nc.scalar.activation(
                        e_sb[:], e_ps[:], AF.Exp, scale=1.0 / 16.0
                    )
                    return e_sb

                # ---- packed q|k projection: one stationary, 16 matmuls ----
                with tc.tile_pool(name="psA", bufs=2, space="PSUM") as psA:
                    for t in range(8):
                        kqp = psA.tile([2 * D, 512], F32, tag="kq_ps")
                        for cb in range(2):
                            nc.tensor.matmul(
                                kqp[:],
                                wqkt[:, cb, :],
                                xsb[:, cb, bass.ts(t, 512)],
                                start=(cb == 0),
                                stop=(cb == 1),
                            )
                        nc.scalar.activation(
                            qt_sb[:, bass.ts(t, 512)], kqp[0:D, :], AF.Copy,
                            accum_out=qacc[:, t : t + 1],
                        )
                        nc.vector.tensor_copy(
                            kt64[D : 2 * D, bass.ts(t, 512)], kqp[D : 2 * D, :]
                        )

                    # center q over n:  qc = q - mean_n(q); first M cols used
                    qsum = wpool.tile([D, 1], F32, tag="qsum")
                    nc.vector.reduce_sum(
                        qsum[:], qacc[:], axis=mybir.AxisListType.X
                    )
                    qneg = wpool.tile([D, 1], F32, tag="qneg")
                    nc.scalar.mul(qneg[:], qsum[:], -1.0 / N)
                    nc.scalar.activation(
                        qtmp[:], qt_sb[:, 0:M], AF.Identity, bias=qneg[:, 0:1]
                    )
                    # scatter k-tiles into DR layout; replicas read the
                    # same sources directly so all transfers are depth-1
                    nc.sync.dma_start(k8[0:16, 0, :], kt64[32:48, :])
                    nc.sync.dma_start(k8[0:16, 1, :], kt64[48:64, :])
                    nc.sync.dma_start(q8[0:16, 0, :], qtmp[0:16, :])
                    nc.sync.dma_start(q8[0:16, 1, :], qtmp[16:32, :])
                    nc.sync.dma_start(k8[32:48, 0, :], kt64[32:48, :])
                    nc.sync.dma_start(k8[32:48, 1, :], kt64[48:64, :])
                    nc.sync.dma_start(q8[32:48, 0, :], qtmp[0:16, :])
                    nc.sync.dma_start(q8[32:48, 1, :], qtmp[16:32, :])

                # ---- chunk 0: V/pm projection rides just-in-time inside
                # ---- the AV loop; vt pair g is produced right before use
                with (
                    tc.tile_pool(name="psO", bufs=1, space="PSUM") as psO,
                    tc.tile_pool(name="psS", bufs=1, space="PSUM") as psS,
                ):

                    def make_acc():
                        o_ps = [
                            psO.tile([128, 512], F32, tag=f"o_ps{ci}",
                                     name=f"o_ps{ci}")
                            for ci in range(2)
                        ]
                        s_ps = psS.tile([1, 512], F32, tag="s_ps", name="s_ps")
                        return o_ps, s_ps

                    def av_group(o_ps, s_ps, g, e_sb):
                        first = g == 0
                        last = g == NG - 1
                        for ci in range(2):
                            nc.tensor.matmul(
                                o_ps[ci][:],
                                vt_sb[:, 2 * g : 2 * g + 2,
                                      128 * ci : 128 * (ci + 1)],
                                e_sb[:],
                                start=first,
                                stop=last,
                                perf_mode=DR,
                            )
                        nc.tensor.matmul(
                            s_ps[:],
                            vt_sb[:, 2 * g : 2 * g + 2, 256:257],
                            e_sb[:],
                            start=first,
                            stop=last,
                            perf_mode=DR,
                        )

                    def tail(mc, o_ps, s_ps, psR):
                        # normalize: out[c,m] = O[c,m] / (VS*s[m]) + vm[c]
                        rs_f = npool.tile([1, 512], F32, tag="rs_f")
                        rs_scr = npool.tile([1, 512], F32, tag="rs_scr")
                        nc.vector.reciprocal_approx_accurate(
                            rs_f[:], s_ps[:], rs_scr[:]
                        )
                        rs_row = npool.tile([1, 512], F32R, tag="rs_row")
                        nc.vector.tensor_copy(rs_row[:], rs_f[:])
                        rb_ps = psR.tile([128, 512], F32, tag="rb_ps",
                                         name="rb_ps")
                        nc.tensor.matmul(
                            rb_ps[:], rsc_row[:], rs_row[:],
                            start=True, stop=True,
                        )
                        rb_sb = npool.tile([128, 512], F32, tag="rb_sb")
                        nc.vector.tensor_copy(rb_sb[:], rb_ps[:])
                        for ci in range(2):
                            t_sb = opool.tile([128, 512], F32, tag="t_sb",
                                              name="t_sb")
                            nc.vector.tensor_mul(
                                t_sb[:], o_ps[ci][:], rb_sb[:]
                            )
                            o_sb = opool.tile([128, 512], F32, tag="o_sb",
                                              name="o_sb")
                            nc.vector.tensor_scalar_add(
                                o_sb[:], t_sb[:], vm_col[:, ci : ci + 1]
                            )
                            nc.sync.dma_start(
                                out_d[128 * ci : 128 * (ci + 1),
                                      bass.ts(mc, 512)],
                                o_sb[:],
                            )

                    o_ps0, s_ps0 = make_acc()
                    with tc.tile_pool(name="psB", bufs=1, space="PSUM") as psB:
                        # bias broadcast tile (bv is added on the Vector
                        # engine)
                        bvb_ps = psB.tile([128, CP], F32, tag="v_ps",
                                          name="bvb")
                        nc.tensor.matmul(
                            bvb_ps[:, 0:257], ones_row_b[:], bvm[:, 0:257],
                            start=True, stop=True,
                        )
                        nc.vector.tensor_copy(bvb[:], bvb_ps[:])

                        for g in range(NG):
                            for h in range(2):
                                jb = 2 * g + h
                                vp = psB.tile([128, CP], F32, tag="v_ps",
                                              name="v_ps")
                                for cb in range(2):
                                    nc.tensor.matmul(
                                        vp[:, 0:257],
                                        xsb[:, cb, bass.ts(jb, 128)],
                                        wvmt[:, cb, 0:257],
                                        start=(cb == 0),
                                        stop=(cb == 1),
                                    )
                                # pm column is pre-bias (bm drops out)
                                nc.scalar.copy(
                                    pmt_col[:, jb : jb + 1], vp[:, 256:257]
                                )
                                vtmp = vpool.tile([128, 256], F32, tag="vtmp",
                                                  name="vtmp")
                                nc.vector.tensor_add(
                                    vtmp[:], vp[:, 0:256], bvb[:, 0:256]
                                )
                                nc.vector.tensor_scalar_max(
                                    vt_sb[:, jb, 0:256], vtmp[:], 0.0
                                )
                            av_group(o_ps0, s_ps0, g, do_exp(emit_e(0, g)))

                    # chunk 1's first energy: keep the Act queue fed while
                    # the mask path runs
                    e_pending = emit_e(1, 0)

                    # ---- mask path: vm[c] = sum_n V^T[n,c] epm[n]/(VS*z) ----
                    with tc.tile_pool(name="psSa", bufs=1,
                                      space="PSUM") as psSa:
                        nc.scalar.activation(
                            epm_col[:, :, 0], pmt_col[:], AF.Exp, scale=RS
                        )
                        sa1 = psSa.tile([128, 512], F32, tag="sa",
                                        name="sa_z")
                        zp = sa1[0:1, 0:NB]
                        nc.tensor.matmul(
                            zp, ones_col_8[:], epm_col[:, :, 0],
                            start=True, stop=True,
                        )
                        zt = wpool.tile([1, 1], F32, tag="zt")
                        nc.vector.reduce_sum(
                            zt[:], zp, axis=mybir.AxisListType.X
                        )
                        rz = wpool.tile([1, 1], F32, tag="rz")
                        nc.vector.reciprocal(rz[:], zt[:])

                        sa2 = psSa.tile([128, 512], F32, tag="sa",
                                        name="sa_vm")
                        vmp = sa2[0:1, 0:C]
                        for g in range(NG):
                            nc.tensor.matmul(
                                vmp,
                                epm_col[:, 2 * g : 2 * g + 2, 0:1],
                                vt_sb[:, 2 * g : 2 * g + 2, 0:256],
                                start=(g == 0),
                                stop=(g == NG - 1),
                                perf_mode=DR,
                            )
                        vm_row = wpool.tile([1, C], F32, tag="vm_row")
                        nc.vector.tensor_scalar(
                            vm_row[:], vmp, rz[0:1, 0:1], RS,
                            mybir.AluOpType.mult, mybir.AluOpType.mult,
                        )
                        sa3 = psSa.tile([128, 512], F32, tag="sa",
                                        name="sa_vc")
                        vcp = sa3[:, 0:2]
                        for ci in range(2):
                            nc.tensor.transpose(
                                vcp[:, ci : ci + 1],
                                vm_row[0:1, 128 * ci : 128 * (ci + 1)],
                                ones_rowf[0:1, 0:1],
                            )
                        nc.vector.tensor_copy(vm_col[:], vcp[:])

                    # ---- tails + software-pipelined chunks 1-3 ----
                    with tc.tile_pool(name="psR", bufs=1,
                                      space="PSUM") as psR:
                        tail(0, o_ps0, s_ps0, psR)
                        for mc in range(1, NMC):
                            o_ps, s_ps = make_acc()
                            for g in range(NG):
                                e_sb = do_exp(e_pending)
                                if g + 1 < NG:
                                    e_pending = emit_e(mc, g + 1)
                                elif mc + 1 < NMC:
                                    e_pending = emit_e(mc + 1, 0)
                                av_group(o_ps, s_ps, g, e_sb)
                            tail(mc, o_ps, s_ps, psR)

    nc.compile()
    return nc


_NC_CACHE = {}


def _get_nc():
    if "nc" not in _NC_CACHE:
        _NC_CACHE["nc"] = build_nc()
    return _NC_CACHE["nc"]


def build_in_maps(x, wq, bq, wk, bk, wv, bv, wm, bm):
    import ml_dtypes

    bf16 = ml_dtypes.bfloat16
    x = np.ascontiguousarray(np.asarray(x, dtype=np.float32))
    xf = x.reshape(B, C, N)

    def blocked(a, dt):
        # [C, F] -> [128, 2, F] (channel block index in dim 1)
        f = a.shape[1]
        return np.ascontiguousarray(
            a.reshape(2, 128, f).transpose(1, 0, 2).astype(dt)
        )

    wqk = np.concatenate(
        [np.asarray(wq, np.float32).T * QS, np.asarray(wk, np.float32).T * QS],
        axis=1,
    )  # [C, 64]: q cols 0-31, k cols 32-63
    wqkt = blocked(wqk, bf16)
    wvm = np.concatenate(
        [
            np.asarray(wv, np.float32).T * VS,
            np.asarray(wm, np.float32).T * VS,
            np.zeros((C, CP - 257), np.float32),
        ],
        axis=1,
    )  # [C, CP]
    wvmt = blocked(wvm, bf16)
    bvm = np.concatenate(
        [np.asarray(bv, np.float32).reshape(C) * VS, np.zeros(CP - C, np.float32)]
    ).reshape(1, CP).astype(bf16)

    in_maps = []
    for core in range(N_CORES):
        b, half = divmod(core, 2)
        if half == 0:
            xin = xf[b]
        else:
            # own query half first; j-sums are permutation invariant
            xin = np.concatenate([xf[b][:, M:], xf[b][:, :M]], axis=1)
        xin = np.ascontiguousarray(
            xin.reshape(2, 128, N).transpose(1, 0, 2).astype(bf16)
        )
        in_maps.append(
            {
                "x": xin,
                "wqkt": wqkt,
                "wvmt": wvmt,
                "bvm": bvm,
            }
        )
    return x, in_maps


def kernel(x, wq, bq, wk, bk, wv, bv, wm, bm):
    x, in_maps = build_in_maps(x, wq, bq, wk, bk, wv, bv, wm, bm)

    res = run_bass_kernel_spmd(_get_nc(), in_maps, list(range(N_CORES)))
    _NC_CACHE["last_results"] = res

    tissue = np.empty((B, C, N), np.float32)
    for core in range(N_CORES):
        b, half = divmod(core, 2)
        tissue[b][:, half * M : (half + 1) * M] = res.results[core]["out"]
    return x, tissue.reshape(B, C, H, W)


# revision 33
# speedup vs baseline: 1.2155x; 1.1788x over previous
"""Trainium2 Bass kernel for the DNL (disentangled non-local) attention block.

Reference computation (per batch b, with xf = x.reshape(B, C, N), N = H*W):
    q  = (wq @ xf + bq)  centered over n          [N, 32]
    k  = (wk @ xf + bk)  centered over n          [32, N]
    A  = softmax_rows(q @ k)                      [N, N]
    v  = relu(wv @ xf + bv)                       [C, N]
    mask = softmax(wm @ xf + bm)                  [N]
    tissue[c, m] = sum_n v[c, n] * (A[m, n] + mask[n])
    return (x, tissue)

Math simplifications used (all exact):
  - q/k biases, bm, and k-centering add per-row constants inside the row
    softmax and drop out; only q-centering survives (as "-mean_n q").
  - The mask term is a rank-1 correction vm[c] = sum_n v[c,n] mask[n].
  - No max-subtraction in softmax: |energy| <= ~5 for these input scales.

Device layout (per core; 8 cores = 4 batches x 2 query-halves of 2048):
  - Projections and attention are a single software-pipelined phase: the
    V/pm projection loop interleaves the E^T matmul + exp for m-chunks
    0-1 (exp run-ahead on a deep fp8 e_sb ring), and chunks 2-3's E+exp
    ride under chunks 0-1's AV matmuls, so the Act engine (exp is the
    per-engine floor at ~55us) never waits on a projection phase.
  - q and k are one packed stationary [wq|wk]; E^T runs fp8 DoubleRow
    (k-tiles of 16) with two j-blocks row-packed at PE row tiles 0/32.
  - exp(E^T) is fp8e4; the AV matmul is fp8 DoubleRow (2 j-blocks =
    K=256 per pass).  The V^T fp8 stationary carries a 257th all-ones
    column: the softmax denominator colsum falls out of the AV passes.
  - Host-side weight scaling keeps fp8 in the normal range: wq,wk x4
    (energy x16, undone by exp scale=1/16), wv x16 (undone via the 1/s
    broadcast row = 1/16 and rz).  wv itself stays bf16: weight-quant
    error does not average out over the attention sum.
  - bv is added by the Vector engine (broadcast tile), not a PE pass.
  - The pm (mask logit) projection rides the V projection as column 256
    of the wvm stationary (pre-bias; bm drops out of its softmax).
  - The per-core query half is selected by permuting the spatial columns
    of the input on the host (j-sums are permutation invariant).
"""

import sys

import numpy as np

if "/opt/trn_rl_repo" not in sys.path:
    sys.path.insert(0, "/opt/trn_rl_repo")

import concourse.bacc as bacc
import concourse.bass as bass
import concourse.mybir as mybir
import concourse.tile as tile
from concourse.bass_utils import run_bass_kernel_spmd

F32 = mybir.dt.float32
F32R = mybir.dt.float32r
BF16 = mybir.dt.bfloat16
FP8 = mybir.dt.float8e4
AF = mybir.ActivationFunctionType
DR = mybir.MatmulPerfMode.DoubleRow

B, C, H, W = 4, 256, 64, 64
N = H * W          # 4096 spatial positions
D = 32             # C // 8, q/k channel dim
M = N // 2         # query rows per core (2048)
NB = N // 128      # 32 j-blocks
NG = NB // 2       # 16 j-block pairs (DoubleRow consumes 2 blocks/pass)
NMC = M // 512     # 4 m-chunks per core
CP = 272           # vt free width: 256 ch + ones col @256, padded to a
                   # 16B-aligned DoubleRow k-tile stride (fp8 Ldweights ISA)
QS = 4.0           # host scale on wq and wk (energy x16)
VS = 16.0          # host scale on wv/bv/wm (v and pm x16)
RS = 1.0 / VS
N_CORES = 8


def build_nc():
    nc = bacc.Bacc("TRN2", target_bir_lowering=False)

    x_d = nc.dram_tensor("x", [128, 2, N], BF16, kind="ExternalInput")
    wqkt_d = nc.dram_tensor("wqkt", [128, 2, 2 * D], BF16, kind="ExternalInput")
    wvmt_d = nc.dram_tensor("wvmt", [128, 2, CP], BF16, kind="ExternalInput")
    bvm_d = nc.dram_tensor("bvm", [1, CP], BF16, kind="ExternalInput")
    out_d = nc.dram_tensor("out", [C, M], F32, kind="ExternalOutput")

    with tile.TileContext(nc) as tc, nc.allow_low_precision(
        reason="bf16/fp8 matmul operands are a deliberate precision/speed trade"
    ):
        with (
            tc.tile_pool(name="const", bufs=1) as cpool,
            tc.tile_pool(name="work", bufs=1) as wpool,
            tc.tile_pool(name="vwork", bufs=3) as vpool,
            tc.tile_pool(name="norm", bufs=2) as npool,
            tc.tile_pool(name="expsb", bufs=36) as epool,
            tc.tile_pool(name="osb", bufs=3) as opool,
        ):
            # ---------------- loads + constants ----------------
            wqkt = cpool.tile([128, 2, 2 * D], BF16, tag="wqkt")
            wvmt = cpool.tile([128, 2, CP], BF16, tag="wvmt")
            bvm = cpool.tile([1, CP], BF16, tag="bvm")
            nc.sync.dma_start(wqkt[:], wqkt_d[:])
            nc.sync.dma_start(wvmt[:], wvmt_d[:])
            nc.sync.dma_start(bvm[:], bvm_d[:])

            xsb = cpool.tile([128, 2, N], BF16, tag="xsb")
            for t in range(4):
                nc.sync.dma_start(
                    xsb[:, :, bass.ts(t, 1024)], x_d[:, :, bass.ts(t, 1024)]
                )

            ones_colf = cpool.tile([128, 1], F32, tag="ones_colf")
            ones_rowf = cpool.tile([1, 128], F32, tag="ones_rowf")
            nc.vector.memset(ones_colf[:], 1.0)
            nc.vector.memset(ones_rowf[:], 1.0)
            ones_row_b = cpool.tile([1, 128], BF16, tag="ones_row_b")
            ones_col_8 = cpool.tile([128, 1], FP8, tag="ones_col_8")
            nc.vector.tensor_copy(ones_row_b[:], ones_rowf[:])
            nc.vector.tensor_copy(ones_col_8[:], ones_colf[:])
            # 1/s broadcast stationary carries the 1/VS unscale of vt
            rsc_f = cpool.tile([1, 128], F32, tag="rsc_f")
            nc.vector.memset(rsc_f[:], RS)
            rsc_row = cpool.tile([1, 128], F32R, tag="rsc_row")
            nc.vector.tensor_copy(rsc_row[:], rsc_f[:])

            # k/q in fp8 DoubleRow layout [16 x 2 k-tiles], replicated to
            # partition group 32 for 2-way row-packed E^T matmuls
            qt_sb = wpool.tile([D, N], F32, tag="qt_sb")
            qacc = wpool.tile([D, 8], F32, tag="qacc")
            kt64 = cpool.tile([64, N], FP8, tag="kt64")
            qtmp = cpool.tile([D, M], FP8, tag="qtmp")
            k8 = cpool.tile([64, 2, N], FP8, tag="k8")
            q8 = cpool.tile([64, 2, M], FP8, tag="q8")
            vt_sb = cpool.tile([128, NB, CP], FP8, tag="vt_sb")
            bvb = cpool.tile([128, CP], BF16, tag="bvb")
            pmt_col = cpool.tile([128, NB], F32, tag="pmt_col")
            epm_col = cpool.tile([128, NB, 16], FP8, tag="epm_col")
            vm_col = cpool.tile([128, 2], F32, tag="vm_col")

            # ones column of V^T (softmax denominator rides the AV matmul)
            nc.vector.memset(vt_sb[:, :, 256:257], 1.0)

            with tc.tile_pool(name="psE", bufs=2, space="PSUM") as psE:

                def emit_e(mc, g):
                    # two K=16x2 fp8-DR matmuls packed at PE row tiles 0/32
                    e_ps = psE.tile([128, 2, 512], F32, tag="e_ps", name="e_ps")
                    for h in range(2):
                        jb = 2 * g + h
                        nc.tensor.matmul(
                            e_ps[:, h, :],
                            k8[h * 32 : h * 32 + 16, :, bass.ts(jb, 128)],
                            q8[h * 32 : h * 32 + 16, :, bass.ts(mc, 512)],
                            start=True,
                            stop=True,
                            perf_mode=DR,
                            tile_position=(h * 32, 0),
                        )
                    return e_ps

                def do_exp(e_ps):
                    e_sb = epool.tile([128, 2, 512], FP8, tag="e_sb",
                                      name="e_sb")
                    # scale=1/16 undoes the host x4 on wq and wk
                    nc.scalar.activation(
                        e_sb[:], e_ps[:], AF.Exp, scale=1.0 / 16.0
                    )
                    return e_sb

                # ---- packed q|k projection: one stationary, 16 matmuls ----
                with tc.tile_pool(name="psA", bufs=4, space="PSUM") as psA:
                    for t in range(8):
                        kqp = psA.tile([2 * D, 512], F32, tag="kq_ps")
                        for cb in range(2):
                            nc.tensor.matmul(
                                kqp[:],
                                wqkt[:, cb, :],
                                xsb[:, cb, bass.ts(t, 512)],
                                start=(cb == 0),
                                stop=(cb == 1),
                            )
                        nc.scalar.activation(
                            qt_sb[:, bass.ts(t, 512)], kqp[0:D, :], AF.Copy,
                            accum_out=qacc[:, t : t + 1],
                        )
                        nc.vector.tensor_copy(
                            kt64[D : 2 * D, bass.ts(t, 512)], kqp[D : 2 * D, :]
                        )

                    # center q over n:  qc = q - mean_n(q); first M cols used
                    qsum = wpool.tile([D, 1], F32, tag="qsum")
                    nc.vector.reduce_sum(
                        qsum[:], qacc[:], axis=mybir.AxisListType.X
                    )
                    qneg = wpool.tile([D, 1], F32, tag="qneg")
                    nc.scalar.mul(qneg[:], qsum[:], -1.0 / N)
                    nc.scalar.activation(
                        qtmp[:], qt_sb[:, 0:M], AF.Identity, bias=qneg[:, 0:1]
                    )
                    # scatter k-tiles into DR layout + replicate to rows 32+
                    nc.sync.dma_start(k8[0:16, 0, :], kt64[32:48, :])
                    nc.sync.dma_start(k8[0:16, 1, :], kt64[48:64, :])
                    nc.sync.dma_start(k8[32:48, :, :], k8[0:16, :, :])
                    nc.sync.dma_start(q8[0:16, 0, :], qtmp[0:16, :])
                    nc.sync.dma_start(q8[0:16, 1, :], qtmp[16:32, :])
                    nc.sync.dma_start(q8[32:48, :, :], q8[0:16, :, :])

                # ---- chunk 0: V/pm projection rides just-in-time inside
                # ---- the AV loop; vt pair g is produced right before use
                with (
                    tc.tile_pool(name="psO", bufs=1, space="PSUM") as psO,
                    tc.tile_pool(name="psS", bufs=1, space="PSUM") as psS,
                ):

                    def make_acc():
                        o_ps = [
                            psO.tile([128, 512], F32, tag=f"o_ps{ci}",
                                     name=f"o_ps{ci}")
                            for ci in range(2)
                        ]
                        s_ps = psS.tile([1, 512], F32, tag="s_ps", name="s_ps")
                        return o_ps, s_ps

                    def av_group(o_ps, s_ps, g, e_sb):
                        first = g == 0
                        last = g == NG - 1
                        for ci in range(2):
                            nc.tensor.matmul(
                                o_ps[ci][:],
                                vt_sb[:, 2 * g : 2 * g + 2,
                                      128 * ci : 128 * (ci + 1)],
                                e_sb[:],
                                start=first,
                                stop=last,
                                perf_mode=DR,
                            )
                        nc.tensor.matmul(
                            s_ps[:],
                            vt_sb[:, 2 * g : 2 * g + 2, 256:257],
                            e_sb[:],
                            start=first,
                            stop=last,
                            perf_mode=DR,
                        )

                    def tail(mc, o_ps, s_ps, psR):
                        # normalize: out[c,m] = O[c,m] / (VS*s[m]) + vm[c]
                        rs_f = npool.tile([1, 512], F32, tag="rs_f")
                        rs_scr = npool.tile([1, 512], F32, tag="rs_scr")
                        nc.vector.reciprocal_approx_accurate(
                            rs_f[:], s_ps[:], rs_scr[:]
                        )
                        rs_row = npool.tile([1, 512], F32R, tag="rs_row")
                        nc.vector.tensor_copy(rs_row[:], rs_f[:])
                        rb_ps = psR.tile([128, 512], F32, tag="rb_ps",
                                         name="rb_ps")
                        nc.tensor.matmul(
                            rb_ps[:], rsc_row[:], rs_row[:],
                            start=True, stop=True,
                        )
                        rb_sb = npool.tile([128, 512], F32, tag="rb_sb")
                        nc.vector.tensor_copy(rb_sb[:], rb_ps[:])
                        for ci in range(2):
                            t_sb = opool.tile([128, 512], F32, tag="t_sb",
                                              name="t_sb")
                            nc.vector.tensor_mul(
                                t_sb[:], o_ps[ci][:], rb_sb[:]
                            )
                            o_sb = opool.tile([128, 512], F32, tag="o_sb",
                                              name="o_sb")
                            nc.vector.tensor_scalar_add(
                                o_sb[:], t_sb[:], vm_col[:, ci : ci + 1]
                            )
                            nc.sync.dma_start(
                                out_d[128 * ci : 128 * (ci + 1),
                                      bass.ts(mc, 512)],
                                o_sb[:],
                            )

                    o_ps0, s_ps0 = make_acc()
                    with tc.tile_pool(name="psB", bufs=1, space="PSUM") as psB:
                        # bias broadcast tile (bv is added on the Vector
                        # engine)
                        bvb_ps = psB.tile([128, CP], F32, tag="v_ps",
                                          name="bvb")
                        nc.tensor.matmul(
                            bvb_ps[:, 0:257], ones_row_b[:], bvm[:, 0:257],
                            start=True, stop=True,
                        )
                        nc.vector.tensor_copy(bvb[:], bvb_ps[:])

                        for g in range(NG):
                            for h in range(2):
                                jb = 2 * g + h
                                vp = psB.tile([128, CP], F32, tag="v_ps",
                                              name="v_ps")
                                for cb in range(2):
                                    nc.tensor.matmul(
                                        vp[:, 0:257],
                                        xsb[:, cb, bass.ts(jb, 128)],
                                        wvmt[:, cb, 0:257],
                                        start=(cb == 0),
                                        stop=(cb == 1),
                                    )
                                # pm column is pre-bias (bm drops out)
                                nc.scalar.copy(
                                    pmt_col[:, jb : jb + 1], vp[:, 256:257]
                                )
                                vtmp = vpool.tile([128, 256], F32, tag="vtmp",
                                                  name="vtmp")
                                nc.vector.tensor_add(
                                    vtmp[:], vp[:, 0:256], bvb[:, 0:256]
                                )
                                nc.vector.tensor_scalar_max(
                                    vt_sb[:, jb, 0:256], vtmp[:], 0.0
                                )
                            av_group(o_ps0, s_ps0, g, do_exp(emit_e(0, g)))

                    # chunk 1's first energy: keep the Act queue fed while
                    # the mask path runs
                    e_pending = emit_e(1, 0)

                    # ---- mask path: vm[c] = sum_n V^T[n,c] epm[n]/(VS*z) ----
                    with tc.tile_pool(name="psSa", bufs=1,
                                      space="PSUM") as psSa:
                        nc.scalar.activation(
                            epm_col[:, :, 0], pmt_col[:], AF.Exp, scale=RS
                        )
                        sa1 = psSa.tile([128, 512], F32, tag="sa",
                                        name="sa_z")
                        zp = sa1[0:1, 0:NB]
                        nc.tensor.matmul(
                            zp, ones_col_8[:], epm_col[:, :, 0],
                            start=True, stop=True,
                        )
                        zt = wpool.tile([1, 1], F32, tag="zt")
                        nc.vector.reduce_sum(
                            zt[:], zp, axis=mybir.AxisListType.X
                        )
                        rz = wpool.tile([1, 1], F32, tag="rz")
                        nc.vector.reciprocal(rz[:], zt[:])

                        sa2 = psSa.tile([128, 512], F32, tag="sa",
                                        name="sa_vm")
                        vmp = sa2[0:1, 0:C]
                        for g in range(NG):
                            nc.tensor.matmul(
                                vmp,
                                epm_col[:, 2 * g : 2 * g + 2, 0:1],
                                vt_sb[:, 2 * g : 2 * g + 2, 0:256],
                                start=(g == 0),
                                stop=(g == NG - 1),
                                perf_mode=DR,
                            )
                        vm_row = wpool.tile([1, C], F32, tag="vm_row")
                        nc.vector.tensor_scalar(
                            vm_row[:], vmp, rz[0:1, 0:1], RS,
                            mybir.AluOpType.mult, mybir.AluOpType.mult,
                        )
                        sa3 = psSa.tile([128, 512], F32, tag="sa",
                                        name="sa_vc")
                        vcp = sa3[:, 0:2]
                        for ci in range(2):
                            nc.tensor.transpose(
                                vcp[:, ci : ci + 1],
                                vm_row[0:1, 128 * ci : 128 * (ci + 1)],
                                ones_rowf[0:1, 0:1],
                            )
                        nc.vector.tensor_copy(vm_col[:], vcp[:])

                    # ---- tails + software-pipelined chunks 1-3 ----
                    with tc.tile_pool(name="psR", bufs=1,
                                      space="PSUM") as psR:
                        tail(0, o_ps0, s_ps0, psR)
                        for mc in range(1, NMC):
                            o_ps, s_ps = make_acc()
                            for g in range(NG):
                                e_sb = do_exp(e_pending)
                                if g + 1 < NG:
                                    e_pending = emit_e(mc, g + 1)
                                elif mc + 1 < NMC:
                                    e_pending = emit_e(mc + 1, 0)
                                av_group(o_ps, s_ps, g, e_sb)
                            tail(mc, o_ps, s_ps, psR)

    nc.compile()
    return nc


_NC_CACHE = {}


def _get_nc():
    if "nc" not in _NC_CACHE:
        _NC_CACHE["nc"] = build_nc()
    return _NC_CACHE["nc"]


def build_in_maps(x, wq, bq, wk, bk, wv, bv, wm, bm):
    import ml_dtypes

    bf16 = ml_dtypes.bfloat16
    x = np.ascontiguousarray(np.asarray(x, dtype=np.float32))
    xf = x.reshape(B, C, N)

    def blocked(a, dt):
        # [C, F] -> [128, 2, F] (channel block index in dim 1)
        f = a.shape[1]
        return np.ascontiguousarray(
            a.reshape(2, 128, f).transpose(1, 0, 2).astype(dt)
        )

    wqk = np.concatenate(
        [np.asarray(wq, np.float32).T * QS, np.asarray(wk, np.float32).T * QS],
        axis=1,
    )  # [C, 64]: q cols 0-31, k cols 32-63
    wqkt = blocked(wqk, bf16)
    wvm = np.concatenate(
        [
            np.asarray(wv, np.float32).T * VS,
            np.asarray(wm, np.float32).T * VS,
            np.zeros((C, CP - 257), np.float32),
        ],
        axis=1,
    )  # [C, CP]
    wvmt = blocked(wvm, bf16)
    bvm = np.concatenate(
        [np.asarray(bv, np.float32).reshape(C) * VS, np.zeros(CP - C, np.float32)]
    ).reshape(1, CP).astype(bf16)

    in_maps = []
    for core in range(N_CORES):
        b, half = divmod(core, 2)
        if half == 0:
            xin = xf[b]
        else:
            # own query half first; j-sums are permutation invariant
            xin = np.concatenate([xf[b][:, M:], xf[b][:, :M]], axis=1)
        xin = np.ascontiguousarray(
            xin.reshape(2, 128, N).transpose(1, 0, 2).astype(bf16)
        )
        in_maps.append(
            {
                "x": xin,
                "wqkt": wqkt,
                "wvmt": wvmt,
                "bvm": bvm,
            }
        )
    return x, in_maps


def kernel(x, wq, bq, wk, bk, wv, bv, wm, bm):
    x, in_maps = build_in_maps(x, wq, bq, wk, bk, wv, bv, wm, bm)

    res = run_bass_kernel_spmd(_get_nc(), in_maps, list(range(N_CORES)))
    _NC_CACHE["last_results"] = res

    tissue = np.empty((B, C, N), np.float32)
    for core in range(N_CORES):
        b, half = divmod(core, 2)
        tissue[b][:, half * M : (half + 1) * M] = res.results[core]["out"]
    return x, tissue.reshape(B, C, H, W)
